# revision 1
# baseline (speedup 1.0000x reference)
"""Trainium2 Bass kernel for AttnBlock:
GroupNorm(32 groups) -> 1x1 q/k/v -> single-head attention over 64x64 tokens
-> 1x1 output projection -> residual.

Sharding: 8 NeuronCores = 2 batches x 4 query-chunks of 1024 tokens (the token
axis is rotated per core on the host, so the program is pure SPMD; key order is
irrelevant to GroupNorm stats, softmax sums, and the attention contraction).
Each core computes GroupNorm + K/V for its batch's full 4096 tokens and
attention + output projection + residual for its 1024 queries.

All matmuls run in float32r (full-rate fp32 streaming, TF32-like operand
rounding, fp32 PSUM accumulation; measured end-to-end rel err ~3e-5).
Softmax runs unnormalized without max-subtraction (scores are O(1) by
construction); the denominator is accumulated on GPSIMD/DVE, all-reduced
across partitions on GPSIMD, and applied after the output projection.
"""
import sys
sys.path.insert(0, '/opt/trn_rl_repo')
from contextlib import ExitStack

import numpy as np
import concourse.bass as bass
import concourse.tile as tile
from concourse import bacc, mybir
from concourse.bass_utils import run_bass_kernel_spmd

F32 = mybir.dt.float32
C = 512
N = 4096
NQ = 1024
KB = 512
NBLK = N // KB
CT = C // 128
QH = NQ // 512
EPS = 1e-6
SCALE = float(np.float32(int(C) ** (-0.5)))
GPSIMD_BCAST = True


def build(mm_dt=mybir.dt.float32r, reps=1):
    nc = bacc.Bacc()
    xb = nc.dram_tensor("xb", [C, N], F32, kind="ExternalInput")
    wqT = nc.dram_tensor("wqT", [C, C], F32, kind="ExternalInput")
    wkT = nc.dram_tensor("wkT", [C, C], F32, kind="ExternalInput")
    wvT = nc.dram_tensor("wvT", [C, C], F32, kind="ExternalInput")
    woT = nc.dram_tensor("woT", [C, C], F32, kind="ExternalInput")
    bq = nc.dram_tensor("bq", [C], F32, kind="ExternalInput")
    bk = nc.dram_tensor("bk", [C], F32, kind="ExternalInput")
    beff = nc.dram_tensor("beff", [C], F32, kind="ExternalInput")
    gamma = nc.dram_tensor("gamma", [C], F32, kind="ExternalInput")
    beta = nc.dram_tensor("beta", [C], F32, kind="ExternalInput")
    gmask = nc.dram_tensor("gmask", [128, 128], F32, kind="ExternalInput")
    out = nc.dram_tensor("out", [C, NQ], F32, kind="ExternalOutput")

    with tile.TileContext(nc) as tc:
     for _rep in range(reps):
      with ExitStack() as ctx:
        const = ctx.enter_context(tc.tile_pool(name="const", bufs=1))
        persist = ctx.enter_context(tc.tile_pool(name="persist", bufs=1))
        ps2 = ctx.enter_context(tc.tile_pool(name="ps2", bufs=2, space="PSUM"))
        ps1 = ctx.enter_context(tc.tile_pool(name="ps1", bufs=1, space="PSUM"))

        h_r = [persist.tile([128, N], mm_dt, tag=f"h{t}", name=f"h{t}") for t in range(CT)]

        def h_ap(t, lo, size):
            return h_r[t][:, lo:lo + size]

        out_acc = persist.tile([128, CT, NQ], F32, tag="out_acc")
        out_r = persist.tile([128, CT, NQ], mm_dt, tag="outr")
        dsum = persist.tile([128, NQ], F32, tag="dsum")
        r_bc = persist.tile([128, NQ], F32, tag="rbc")

        with tc.tile_pool(name="xpool", bufs=1) as xpool, \
             tc.tile_pool(name="gtmp", bufs=1) as gtmp, \
             tc.tile_pool(name="wstage", bufs=3) as wstage_pool:
            # ---- x loads first: two half-tiles per channel tile ----
            xh = []
            for t in range(CT):
                halves = []
                for hh in range(2):
                    xt = xpool.tile([128, N // 2], F32, tag=f"x{t}h{hh}", name=f"x{t}h{hh}")
                    nc.sync.dma_start(
                        xt[:], xb[t * 128:(t + 1) * 128, hh * (N // 2):(hh + 1) * (N // 2)])
                    halves.append(xt)
                xh.append(halves)

            # ---- constants while x streams ----
            gmask_sb = const.tile([128, 128], F32, tag="gmask")
            nc.sync.dma_start(gmask_sb[:], gmask[:, :])
            eps_sb = const.tile([128, 1], F32, tag="eps")
            nc.vector.memset(eps_sb[:], EPS)
            nc.vector.memset(out_acc[:], 0.0)
            nc.vector.memset(dsum[:], 0.0)

            def load_cvec(t, tagname):
                sb = const.tile([128, CT], F32, tag=tagname, name=tagname)
                nc.sync.dma_start(sb[:], t[:].rearrange("(t p) -> p t", p=128))
                return sb

            gamma_sb = load_cvec(gamma, "gamma")
            beta_sb = load_cvec(beta, "beta")
            bq_sb = load_cvec(bq, "bq")
            bk_sb = load_cvec(bk, "bk")
            beff_sb = load_cvec(beff, "beff")

            # weight DMAs queue behind x; rounding copies run on ScalarE in a
            # controlled order so neither DVE nor ACT stalls on weight DMAs
            # ahead of GroupNorm work.
            w_stage = {}
            for wname, wdram in (("k", wkT), ("v", wvT), ("q", wqT), ("o", woT)):
                tiles = []
                for kt in range(CT):
                    st = wstage_pool.tile([128, C], F32, tag="wst",
                                          name=f"wst{wname}{kt}")
                    nc.sync.dma_start(st[:], wdram[kt * 128:(kt + 1) * 128, :])
                    tiles.append(st)
                w_stage[wname] = tiles
            w_r = {}

            def round_weights(wname):
                tiles = []
                for kt in range(CT):
                    wr = const.tile([128, C], mm_dt, tag=f"w{wname}{kt}", name=f"w{wname}{kt}")
                    nc.scalar.activation(out=wr[:], in_=w_stage[wname][kt][:],
                                         func=mybir.ActivationFunctionType.Copy,
                                         scale=1.0)
                    tiles.append(wr)
                w_r[wname] = tiles

            # ---- GroupNorm stats: per-tile bn_stats, one combined mask-matmul ----
            ab_sb = gtmp.tile([128, CT, 2], F32, tag="ab")  # [:,t,0]=a, [:,t,1]=nb
            stk = gtmp.tile([128, 2 * CT], F32, tag="stk")  # cols t: mean, CT+t: E[x^2]
            for t in range(CT):
                stats = gtmp.tile([128, 8, 6], F32, tag="bst", name=f"bst{t}")
                for i in range(8):
                    src_ = xh[t][i // 4][:, (i % 4) * 512:(i % 4 + 1) * 512]
                    nc.vector.bn_stats(out=stats[:, i, :], in_=src_)
                mv = gtmp.tile([128, 2], F32, tag="mv", name=f"mv{t}")
                nc.vector.bn_aggr(out=mv[:], in_=stats[:])
                nc.vector.tensor_copy(stk[:, t:t + 1], mv[:, 0:1])
                nc.vector.tensor_mul(stk[:, CT + t:CT + t + 1], mv[:, 0:1], mv[:, 0:1])
                nc.vector.tensor_add(stk[:, CT + t:CT + t + 1], stk[:, CT + t:CT + t + 1], mv[:, 1:2])
            psg = ps2.tile([128, 512], F32, tag="pk", name="psg", bufs=3)
            nc.tensor.matmul(psg[:, :2 * CT], gmask_sb[:], stk[:], start=True, stop=True)
            mean_sb = gtmp.tile([128, CT], F32, tag="mean")
            nc.vector.tensor_copy(mean_sb[:], psg[:, 0:CT])
            var_sb = gtmp.tile([128, CT], F32, tag="var")
            nc.vector.tensor_mul(var_sb[:], mean_sb[:], mean_sb[:])
            nc.vector.tensor_tensor(var_sb[:], psg[:, CT:2 * CT], var_sb[:], mybir.AluOpType.subtract)
            nc.scalar.activation(out=var_sb[:], in_=var_sb[:],
                                 func=mybir.ActivationFunctionType.Sqrt,
                                 bias=eps_sb[:], scale=1.0)
            nc.vector.reciprocal(var_sb[:], var_sb[:])
            for t in range(CT):
                nc.vector.tensor_mul(ab_sb[:, t, 0:1], var_sb[:, t:t + 1], gamma_sb[:, t:t + 1])
                nc.vector.tensor_mul(var_sb[:, t:t + 1], mean_sb[:, t:t + 1], ab_sb[:, t, 0:1])
                nc.vector.tensor_tensor(ab_sb[:, t, 1:2], beta_sb[:, t:t + 1], var_sb[:, t:t + 1],
                                        mybir.AluOpType.subtract)

            wsched = {0: ["k"], 2: ["v", "q"], 4: ["o"]}
            for ch in range(8):
                for wn in wsched.get(ch, []):
                    round_weights(wn)
                for t in range(CT):
                    sl = xh[t][ch // 4][:, (ch % 4) * 512:(ch % 4 + 1) * 512]
                    if ch < 2:
                        nc.scalar.activation(
                            out=h_r[t][:, ch * 512:(ch + 1) * 512], in_=sl,
                            func=mybir.ActivationFunctionType.Identity,
                            bias=ab_sb[:, t, 1:2], scale=ab_sb[:, t, 0:1])
                    else:
                        nc.vector.tensor_scalar(
                            out=h_r[t][:, ch * 512:(ch + 1) * 512], in0=sl,
                            scalar1=ab_sb[:, t, 0:1], scalar2=ab_sb[:, t, 1:2],
                            op0=mybir.AluOpType.mult, op1=mybir.AluOpType.add)

        # ---- attention over key blocks ----
        with tc.tile_pool(name="blk", bufs=2) as blk:
            q_sb = blk.tile([128, CT, NQ], mm_dt, tag="q", bufs=1)
            def emit_kv(b):
                ko = b * KB
                k_blk = blk.tile([128, CT, KB], mm_dt, tag="kblk", name=f"kblk{b}")
                for ct in range(CT):
                    pk = ps2.tile([128, 512], F32, tag="pk", name=f"pk{b}{ct}", bufs=3)
                    for kt in range(CT):
                        nc.tensor.matmul(pk[:], w_r["k"][kt][:, ct * 128:(ct + 1) * 128],
                                         h_ap(kt, ko, KB),
                                         start=(kt == 0), stop=(kt == CT - 1))
                    nc.scalar.activation(out=k_blk[:, ct, :], in_=pk[:],
                                         func=mybir.ActivationFunctionType.Identity,
                                         bias=bk_sb[:, ct:ct + 1], scale=1.0)

                vt_blk = blk.tile([128, CT, C], mm_dt, tag="vtblk", name=f"vtblk{b}")
                for kc in range(CT):
                    pv = ps2.tile([128, 512], F32, tag="pk", name=f"pv{b}{kc}", bufs=3)
                    for kt in range(CT):
                        nc.tensor.matmul(pv[:], h_ap(kt, ko + kc * 128, 128),
                                         w_r["v"][kt][:],
                                         start=(kt == 0), stop=(kt == CT - 1))
                    nc.vector.tensor_copy(vt_blk[:, kc, :], pv[:])

                return k_blk, vt_blk

            def emit_attn(b, k_blk, vt_blk):
                ko = b * KB
                for qh in range(QH):
                    at_q = blk.tile([128, CT, 512], mm_dt, tag="atblk", name=f"at{b}{qh}")
                    for kc in range(CT):
                        pst = ps2.tile([128, 512], F32, tag="ps_s", name=f"pst{b}{kc}{qh}", bufs=3)
                        for ct in range(CT):
                            nc.tensor.matmul(pst[:], k_blk[:, ct, kc * 128:(kc + 1) * 128],
                                             q_sb[:, ct, qh * 512:(qh + 1) * 512],
                                             start=(ct == 0), stop=(ct == CT - 1))
                        nc.scalar.activation(out=at_q[:, kc, :], in_=pst[:],
                                             func=mybir.ActivationFunctionType.Exp,
                                             scale=SCALE)
                    for kc in range(CT):
                        if b == NBLK - 1:
                            nc.vector.tensor_add(dsum[:, qh * 512:(qh + 1) * 512],
                                                 dsum[:, qh * 512:(qh + 1) * 512],
                                                 at_q[:, kc, :])
                        else:
                            nc.gpsimd.tensor_tensor(dsum[:, qh * 512:(qh + 1) * 512],
                                                    dsum[:, qh * 512:(qh + 1) * 512],
                                                    at_q[:, kc, :], mybir.AluOpType.add)
                    if b == NBLK - 1:
                        from concourse import bass_isa
                        sl = slice(qh * 512, (qh + 1) * 512)
                        nc.gpsimd.partition_all_reduce(
                            r_bc[:, sl], dsum[:, sl], channels=128,
                            reduce_op=bass_isa.ReduceOp.add)
                        nc.vector.reciprocal(r_bc[:, sl], r_bc[:, sl])
                    for ct in range(CT):
                        pav = ps2.tile([128, 512], F32, tag="pav", name=f"pav{b}{ct}{qh}")
                        for kc in range(CT):
                            nc.tensor.matmul(pav[:], vt_blk[:, kc, ct * 128:(ct + 1) * 128],
                                             at_q[:, kc, :],
                                             start=(kc == 0), stop=(kc == CT - 1))
                        if b == NBLK - 1:
                            nc.vector.tensor_tensor(
                                out_r[:, ct, qh * 512:(qh + 1) * 512],
                                out_acc[:, ct, qh * 512:(qh + 1) * 512], pav[:],
                                mybir.AluOpType.add)
                        else:
                            nc.vector.tensor_add(out_acc[:, ct, qh * 512:(qh + 1) * 512],
                                                 out_acc[:, ct, qh * 512:(qh + 1) * 512], pav[:])

            kv = emit_kv(0)

            # ---- Q projection (after block-0 K/V so PE isn't stream-stalled
            # waiting for wq while wk-dependent work is ready) ----
            for ct in range(CT):
                for qh in range(QH):
                    pq = ps2.tile([128, 512], F32, tag="pk", name=f"pq{ct}{qh}", bufs=3)
                    for kt in range(CT):
                        nc.tensor.matmul(pq[:], w_r["q"][kt][:, ct * 128:(ct + 1) * 128],
                                         h_ap(kt, qh * 512, 512),
                                         start=(kt == 0), stop=(kt == CT - 1))
                    nc.scalar.activation(out=q_sb[:, ct, qh * 512:(qh + 1) * 512], in_=pq[:],
                                         func=mybir.ActivationFunctionType.Identity,
                                         bias=bq_sb[:, ct:ct + 1], scale=1.0)

            for b in range(NBLK):
                nxt = emit_kv(b + 1) if b + 1 < NBLK else None
                emit_attn(b, *kv)
                kv = nxt
        # ---- epilogue ----
        with tc.tile_pool(name="epi", bufs=8) as epi, \
             tc.tile_pool(name="epi1", bufs=1) as epi1:
            from concourse import bass_isa
            xres = epi1.tile([128, CT, NQ], F32, tag="xres")
            for t in range(CT):
                nc.sync.dma_start(xres[:, t, :], xb[t * 128:(t + 1) * 128, 0:NQ])
            for t in range(CT):
                nc.scalar.activation(out=xres[:, t, :], in_=xres[:, t, :],
                                     func=mybir.ActivationFunctionType.Identity,
                                     bias=beff_sb[:, t:t + 1], scale=1.0)

            for qh in range(QH):
                for ct in range(CT):
                    pp = ps2.tile([128, 512], F32, tag="pk", name=f"pp{ct}{qh}", bufs=3)
                    for kt in range(CT):
                        nc.tensor.matmul(pp[:], w_r["o"][kt][:, ct * 128:(ct + 1) * 128],
                                         out_r[:, kt, qh * 512:(qh + 1) * 512],
                                         start=(kt == 0), stop=(kt == CT - 1))
                    ot = epi.tile([128, 512], F32, tag="ot", name=f"ot{ct}{qh}")
                    nc.vector.tensor_mul(ot[:], pp[:], r_bc[:, qh * 512:(qh + 1) * 512])
                    nc.vector.tensor_add(ot[:], ot[:], xres[:, ct, qh * 512:(qh + 1) * 512])
                    nc.sync.dma_start(out[ct * 128:(ct + 1) * 128, qh * 512:(qh + 1) * 512], ot[:])

    nc.compile()
    return nc


def make_in_maps(x, gn_gamma, gn_beta, wq, bq, wk, bk, wv, bv, wo, bo):
    B = x.shape[0]
    xf = np.ascontiguousarray(x.reshape(B, C, N).astype(np.float32))
    base = {
        "wqT": np.ascontiguousarray(wq.T.astype(np.float32)),
        "wkT": np.ascontiguousarray(wk.T.astype(np.float32)),
        "wvT": np.ascontiguousarray(wv.T.astype(np.float32)),
        "woT": np.ascontiguousarray(wo.T.astype(np.float32)),
        "bq": np.asarray(bq, np.float32),
        "bk": np.asarray(bk, np.float32),
        "beff": np.asarray(bo, np.float32) + np.asarray(wo, np.float32) @ np.asarray(bv, np.float32),
        "gamma": np.asarray(gn_gamma, np.float32),
        "beta": np.asarray(gn_beta, np.float32),
        "gmask": _gmask(),
    }
    in_maps = []
    for i in range(8):
        b, qc = i // 4, i % 4
        qoff = qc * NQ
        xrot = np.roll(xf[b], -qoff, axis=1)
        in_maps.append({**base, "xb": np.ascontiguousarray(xrot)})
    return in_maps


def _gmask():
    m = np.zeros((128, 128), np.float32)
    gs = 16
    for g in range(128 // gs):
        m[g * gs:(g + 1) * gs, g * gs:(g + 1) * gs] = 1.0 / gs
    return m


def assemble(results):
    full = np.zeros((2, C, N), np.float32)
    for i in range(8):
        b, qc = i // 4, i % 4
        full[b][:, qc * NQ:(qc + 1) * NQ] = results[i]["out"]
    return full.reshape(2, C, 64, 64)


_NC_CACHE = {}


def kernel(**inputs):
    import numpy as np
    x = np.asarray(inputs["x"], np.float32)
    if "build" not in _NC_CACHE:
        _NC_CACHE["build"] = build()
    nc = _NC_CACHE["build"]
    in_maps = make_in_maps(
        x, inputs["gn_gamma"], inputs["gn_beta"],
        inputs["wq"], inputs["bq"], inputs["wk"], inputs["bk"],
        inputs["wv"], inputs["bv"], inputs["wo"], inputs["bo"])
    res = run_bass_kernel_spmd(nc, in_maps, core_ids=list(range(8)))
    return assemble(res.results)



# revision 2
# speedup vs baseline: 1.6968x; 1.6968x over previous
"""Trainium2 Bass kernel for AttnBlock (fp8e4m3 DoubleRow version):
GroupNorm(32 groups) -> 1x1 q/k/v -> single-head attention over 64x64 tokens
-> 1x1 output projection -> residual.

Sharding: 8 NeuronCores = 2 batches x 4 query-chunks of 1024 tokens (token
axis rotated per core on the host; pure SPMD). Each core computes GroupNorm +
K/V for its batch's full 4096 tokens and attention + output projection +
residual for its 1024 queries.

All big matmuls run in fp8e4m3 with perf_mode=DoubleRow (2 MACs/cell/cycle,
0.5 cycles per output column): operands carry contraction pairs along a
[128, 2, free] access-pattern dim. x ships as bf16 and the four weight
matrices ship pre-converted to fp8 from the host, cutting input DMA from
12MB to 5MB. Softmax is unnormalized: at = exp(s*scale - ln8) rounded to
fp8 (<< 240 = TRN fp8e4 max). The denominator comes from an all-ones
stationary DoubleRow matmul over the exp'd scores, which also broadcasts the
per-query sums across partitions for free; ln8 cancels in the normalize.
The residual uses bf16 x, so fp8 noise only rides on the small attention
output (~40x smaller than x); end-to-end rel err ~6e-3 vs the 2e-2 gate.
"""
import sys
sys.path.insert(0, '/opt/trn_rl_repo')
from contextlib import ExitStack

import numpy as np
import concourse.bass as bass
import concourse.tile as tile
from concourse import bacc, mybir
from concourse.bass_utils import run_bass_kernel_spmd

F32 = mybir.dt.float32
BF16 = mybir.dt.bfloat16
F8 = mybir.dt.float8e4
DR = mybir.MatmulPerfMode.DoubleRow
AF = mybir.ActivationFunctionType
OP = mybir.AluOpType

C = 512
N = 4096
NQ = 1024
CT = C // 128    # 4 channel tiles
KT = N // 128    # 32 key tiles
TB = N // 512    # 8 token blocks of 512
QH = NQ // 512   # 2 query halves
EPS = 1e-6
SCALE = float(np.float32(int(C) ** (-0.5)))
EXPB = -float(np.log(8.0))


def build(reps=1):
    nc = bacc.Bacc()
    xb = nc.dram_tensor("xb", [C, N], BF16, kind="ExternalInput")
    wqk8 = nc.dram_tensor("wqk8", [C, C], F8, kind="ExternalInput")
    wv8 = nc.dram_tensor("wv8", [C, C], F8, kind="ExternalInput")
    wo8 = nc.dram_tensor("wo8", [C, C], F8, kind="ExternalInput")
    bqk = nc.dram_tensor("bqk", [C], F32, kind="ExternalInput")
    beff = nc.dram_tensor("beff", [C], F32, kind="ExternalInput")
    gamma = nc.dram_tensor("gamma", [C], F32, kind="ExternalInput")
    beta = nc.dram_tensor("beta", [C], F32, kind="ExternalInput")
    gmask = nc.dram_tensor("gmask", [128, 128], F32, kind="ExternalInput")
    out = nc.dram_tensor("out", [C, NQ], F32, kind="ExternalOutput")

    with tile.TileContext(nc) as tc:
     for _rep in range(reps):
      with ExitStack() as ctx:
        const = ctx.enter_context(tc.tile_pool(name="const", bufs=1))
        persist = ctx.enter_context(tc.tile_pool(name="persist", bufs=1))

        h3 = persist.tile([128, CT, N], F8, tag="h3")
        vt3 = persist.tile([128, KT, C], F8, tag="vt3")
        qk3 = persist.tile([128, CT, NQ], F8, tag="qk3")
        at3 = persist.tile([128, KT, QH, 512], F8, tag="at3")
        or3 = persist.tile([128, CT, NQ], F8, tag="or3")
        rbc = persist.tile([128, QH, 512], F32, tag="rbc")
        xres = persist.tile([128, CT, NQ], BF16, tag="xres")

        with tc.tile_pool(name="xpool", bufs=1) as xpool, \
             tc.tile_pool(name="gtmp", bufs=1) as gtmp, \
             tc.tile_pool(name="psp", bufs=2, space="PSUM") as psp, \
             tc.tile_pool(name="psd", bufs=1, space="PSUM") as psd, \
             tc.tile_pool(name="pss", bufs=2, space="PSUM") as pss:
            # ---- input DMAs: wk first (needed earliest), then x, wq, wv, wo
            w3 = {wn: const.tile([128, CT, C], F8, tag=f"w3{wn}", name=f"w3{wn}")
                  for wn in ("qk", "v", "o")}

            def load_w(wn, wdram):
                for kt in range(CT):
                    nc.sync.dma_start(w3[wn][:, kt, :],
                                      wdram[kt * 128:(kt + 1) * 128, :])

            load_w("qk", wqk8)

            xh = []
            for t in range(CT):
                halves = []
                for hh in range(2):
                    xt = xpool.tile([128, N // 2], BF16, tag=f"x{t}h{hh}",
                                    name=f"x{t}h{hh}")
                    # alternate the two HWDGE queues so x streams in parallel
                    eng = nc.sync if (2 * t + hh) % 2 == 0 else nc.scalar
                    eng.dma_start(
                        xt[:], xb[t * 128:(t + 1) * 128,
                                  hh * (N // 2):(hh + 1) * (N // 2)])
                    halves.append(xt)
                xh.append(halves)

            def xsl(t, ch):
                return xh[t][ch // 4][:, (ch % 4) * 512:(ch % 4 + 1) * 512]

            load_w("v", wv8)
            load_w("o", wo8)

            # ---- constants while x streams ----
            gmask_sb = const.tile([128, 128], F32, tag="gmask")
            nc.sync.dma_start(gmask_sb[:], gmask[:, :])
            eps_sb = const.tile([128, 1], F32, tag="eps")
            nc.vector.memset(eps_sb[:], EPS)
            ones2 = const.tile([128, 2, 128], F8, tag="ones2")
            nc.vector.memset(ones2[:], 1.0)
            expb_sb = const.tile([128, 1], F32, tag="expb")
            nc.vector.memset(expb_sb[:], EXPB)

            def load_cvec(t, tagname):
                sb = const.tile([128, CT], F32, tag=tagname, name=tagname)
                nc.sync.dma_start(sb[:], t[:].rearrange("(t p) -> p t", p=128))
                return sb

            gamma_sb = load_cvec(gamma, "gamma")
            beta_sb = load_cvec(beta, "beta")
            bqk_sb = load_cvec(bqk, "bqk")
            beff_sb = load_cvec(beff, "beff")

            # ---- GroupNorm stats. stk[:, t] = E[x], stk[:, CT+t] = E[x^2]
            # per partition-channel. t=0..2 via bn_stats on DVE; t=3 via
            # Identity/Square accum_out passes on the otherwise-idle ACT so
            # the two engines finish the stats chain together. ----
            ab_sb = gtmp.tile([128, CT, 2], F32, tag="ab")
            stk = gtmp.tile([128, 2 * CT], F32, tag="stk")
            for t in range(1, 4):
                stats = gtmp.tile([128, 8, 6], F32, tag="bst", name=f"bst{t}")
                for i in range(8):
                    nc.vector.bn_stats(out=stats[:, i, :], in_=xsl(t, i))
                mv = gtmp.tile([128, 2], F32, tag="mv", name=f"mv{t}")
                nc.vector.bn_aggr(out=mv[:], in_=stats[:])
                nc.vector.tensor_copy(stk[:, t:t + 1], mv[:, 0:1])
                nc.vector.tensor_mul(stk[:, CT + t:CT + t + 1], mv[:, 0:1], mv[:, 0:1])
                nc.vector.tensor_add(stk[:, CT + t:CT + t + 1],
                                     stk[:, CT + t:CT + t + 1], mv[:, 1:2])
            t = 0
            sacc = gtmp.tile([128, 2, 8], F32, tag="sacc")
            for i in range(8):
                sl = xsl(t, i)
                jk = gtmp.tile([128, 512], F32, tag="sjunk", name=f"sj{i}",
                               bufs=2)
                nc.scalar.activation(out=jk[:], in_=sl, func=AF.Identity,
                                     scale=1.0, accum_out=sacc[:, 0, i:i + 1])
                nc.scalar.activation(out=jk[:], in_=sl, func=AF.Square,
                                     scale=1.0, accum_out=sacc[:, 1, i:i + 1])
            nc.vector.tensor_reduce(out=stk[:, t:t + 1], in_=sacc[:, 0, :],
                                    axis=mybir.AxisListType.X, op=OP.add)
            nc.vector.tensor_reduce(out=stk[:, CT + t:CT + t + 1],
                                    in_=sacc[:, 1, :],
                                    axis=mybir.AxisListType.X, op=OP.add)
            nc.vector.tensor_scalar(out=stk[:, t:t + 1], in0=stk[:, t:t + 1],
                                    scalar1=1.0 / N, scalar2=None,
                                    op0=OP.mult, op1=OP.bypass)
            nc.vector.tensor_scalar(out=stk[:, CT + t:CT + t + 1],
                                    in0=stk[:, CT + t:CT + t + 1],
                                    scalar1=1.0 / N, scalar2=None,
                                    op0=OP.mult, op1=OP.bypass)
            psg = psp.tile([128, 512], F32, tag="pp", name="psg")
            nc.tensor.matmul(psg[:, :2 * CT], gmask_sb[:], stk[:],
                             start=True, stop=True)
            mean_sb = gtmp.tile([128, CT], F32, tag="mean")
            nc.vector.tensor_copy(mean_sb[:], psg[:, 0:CT])
            var_sb = gtmp.tile([128, CT], F32, tag="var")
            nc.vector.tensor_mul(var_sb[:], mean_sb[:], mean_sb[:])
            nc.vector.tensor_tensor(var_sb[:], psg[:, CT:2 * CT], var_sb[:],
                                    OP.subtract)
            nc.scalar.activation(out=var_sb[:], in_=var_sb[:], func=AF.Sqrt,
                                 bias=eps_sb[:], scale=1.0)
            nc.vector.reciprocal(var_sb[:], var_sb[:])
            for t in range(CT):
                nc.vector.tensor_mul(ab_sb[:, t, 0:1], var_sb[:, t:t + 1],
                                     gamma_sb[:, t:t + 1])
                nc.vector.tensor_mul(var_sb[:, t:t + 1], mean_sb[:, t:t + 1],
                                     ab_sb[:, t, 0:1])
                nc.vector.tensor_tensor(ab_sb[:, t, 1:2], beta_sb[:, t:t + 1],
                                        var_sb[:, t:t + 1], OP.subtract)

            # ---- h3 (fp8 GroupNorm output), emitted in token order.
            # First chunks on ACT+DVE for latency (Q proj and early K/V need
            # them); later chunks on the otherwise-idle Pool engine. ----
            def emit_h(ch, fast):
                for t in range(CT):
                    sl = xsl(t, ch)
                    if fast:
                        eng = nc.scalar if (t + ch) % 2 else nc.vector
                    else:
                        eng = nc.gpsimd
                    if eng is nc.scalar:
                        nc.scalar.activation(
                            out=h3[:, t, ch * 512:(ch + 1) * 512], in_=sl,
                            func=AF.Identity,
                            bias=ab_sb[:, t, 1:2], scale=ab_sb[:, t, 0:1])
                    else:
                        eng.tensor_scalar(
                            out=h3[:, t, ch * 512:(ch + 1) * 512], in0=sl,
                            scalar1=ab_sb[:, t, 0:1], scalar2=ab_sb[:, t, 1:2],
                            op0=OP.mult, op1=OP.add)

            emit_h(0, True)
            emit_h(1, True)

            # ---- qk = Wqk h + bqk over the 1024 query tokens, where
            # Wqk = Wk^T Wq and bqk = Wk^T bq are folded on the host.
            # scores = (Wk h_k + bk)*(Wq h_q + bq) = h_k*(Wqk h_q + bqk)
            # + bk*(...), and the bk term is constant per query column so it
            # cancels in the softmax. Neither K nor Q is ever materialized. --
            for ct in range(CT):
                for qh in range(QH):
                    pqk = psp.tile([128, 512], F32, tag="pp",
                                   name=f"pqk{ct}{qh}")
                    for i in range(CT // 2):
                        nc.tensor.matmul(
                            pqk[:], w3["qk"][:, 2 * i:2 * i + 2,
                                             ct * 128:(ct + 1) * 128],
                            h3[:, 2 * i:2 * i + 2, qh * 512:(qh + 1) * 512],
                            start=(i == 0), stop=(i == CT // 2 - 1),
                            perf_mode=DR)
                    nc.vector.tensor_scalar(
                        out=qk3[:, ct, qh * 512:(qh + 1) * 512], in0=pqk[:],
                        scalar1=bqk_sb[:, ct:ct + 1], scalar2=None,
                        op0=OP.add, op1=OP.bypass)

            # ---- pipelined token-block loop: h3 -> V -> scores -> exp,
            # with the qh=0 half of AV accumulating in-loop (4 banks) as
            # exp'd key-tile pairs complete. AV is evicted UNNORMALIZED
            # (scaled by 1/512 to sit in fp8 range); the per-query softmax
            # denominator is applied after the O-projection instead, which
            # is exact since both are linear per query column. ----
            dps = psd.tile([128, QH, 512], F32, tag="dps")
            for tb in range(TB):
                if tb + 2 < TB:
                    emit_h(tb + 2, tb < 2)
                if tb == 5:
                    # xres: SBUF copy of x[:, :NQ] for the residual, on Pool
                    # after its h3 queue has drained (only needed at epilogue)
                    for t in range(CT):
                        for qh in range(QH):
                            nc.gpsimd.tensor_copy(
                                xres[:, t, qh * 512:(qh + 1) * 512],
                                xsl(t, qh))
                # scores+exp and V-projection interleaved: the V matmuls keep
                # the PE busy while scores wait for the exp double-buffer
                for kk in range(4):
                    kt_ = tb * 4 + kk
                    ps2 = pss.tile([128, QH, 512], F32, tag="ps2",
                                   name=f"ps2_{kt_}")
                    for i in range(CT // 2):
                        for qh in range(QH):
                            nc.tensor.matmul(
                                ps2[:, qh, :],
                                h3[:, 2 * i:2 * i + 2,
                                   kt_ * 128:(kt_ + 1) * 128],
                                qk3[:, 2 * i:2 * i + 2,
                                    qh * 512:(qh + 1) * 512],
                                start=(i == 0), stop=(i == CT // 2 - 1),
                                perf_mode=DR)
                    nc.scalar.activation(out=at3[:, kt_, :, :], in_=ps2[:],
                                         func=AF.Exp, bias=expb_sb[:],
                                         scale=SCALE)
                    # V projection (transposed out: stationary = h3 key slice)
                    pv = psp.tile([128, 512], F32, tag="pp", name=f"pv{kt_}")
                    for i in range(CT // 2):
                        nc.tensor.matmul(
                            pv[:], h3[:, 2 * i:2 * i + 2,
                                      kt_ * 128:(kt_ + 1) * 128],
                            w3["v"][:, 2 * i:2 * i + 2, :],
                            start=(i == 0), stop=(i == CT // 2 - 1),
                            perf_mode=DR)
                    nc.vector.tensor_copy(vt3[:, kt_, :], pv[:])
                    # dsum[q] = sum_k at[k, q] via the all-ones stationary,
                    # which also broadcasts the sums across all partitions
                    if kk % 2:
                        i = (tb * 4 + kk) // 2
                        for qh in range(QH):
                            nc.tensor.matmul(
                                dps[:, qh, :], ones2[:],
                                at3[:, 2 * i:2 * i + 2, qh, :],
                                start=(i == 0), stop=(i == KT // 2 - 1),
                                perf_mode=DR)
            for qh in range(QH):
                nc.vector.reciprocal(rbc[:, qh, :], dps[:, qh, :])

        # ---- attention epilogue: AV, O-proj, residual ----
        with tc.tile_pool(name="psav", bufs=8, space="PSUM") as psav, \
             tc.tile_pool(name="epi", bufs=4) as epi:
            # AV per query-half: out[c, q] = sum_k vt[k, c] * at[k, q].
            # qh0's O-projection + store overlap qh1's AV matmuls.
            for qh in range(QH):
                pav = [psav.tile([128, 512], F32, tag="pav",
                                 name=f"pav{ct}_{qh}") for ct in range(CT)]
                for i in range(KT // 2):
                    for ct in range(CT):
                        nc.tensor.matmul(
                            pav[ct][:],
                            vt3[:, 2 * i:2 * i + 2, ct * 128:(ct + 1) * 128],
                            at3[:, 2 * i:2 * i + 2, qh, :],
                            start=(i == 0), stop=(i == KT // 2 - 1),
                            perf_mode=DR)
                for ct in range(CT):
                    nc.vector.tensor_tensor(
                        or3[:, ct, qh * 512:(qh + 1) * 512],
                        pav[ct][:], rbc[:, qh, :], OP.mult)
                # O projection + bias + residual, then store
                for ct in range(CT):
                    pp = psav.tile([128, 512], F32, tag="pav",
                                   name=f"po{ct}{qh}")
                    for i in range(CT // 2):
                        nc.tensor.matmul(
                            pp[:], w3["o"][:, 2 * i:2 * i + 2,
                                           ct * 128:(ct + 1) * 128],
                            or3[:, 2 * i:2 * i + 2, qh * 512:(qh + 1) * 512],
                            start=(i == 0), stop=(i == CT // 2 - 1),
                            perf_mode=DR)
                    ot = epi.tile([128, 512], F32, tag="ot", name=f"ot{ct}{qh}")
                    nc.vector.scalar_tensor_tensor(
                        out=ot[:], in0=pp[:], scalar=beff_sb[:, ct:ct + 1],
                        in1=xres[:, ct, qh * 512:(qh + 1) * 512],
                        op0=OP.add, op1=OP.add)
                    eng = nc.sync if ct % 2 == 0 else nc.scalar
                    eng.dma_start(
                        out[ct * 128:(ct + 1) * 128, qh * 512:(qh + 1) * 512],
                        ot[:])

    nc.compile()
    return nc


F8NP = mybir.dt.np(F8)
BF16NP = mybir.dt.np(BF16)


def make_in_maps(x, gn_gamma, gn_beta, wq, bq, wk, bk, wv, bv, wo, bo):
    B = x.shape[0]
    xf = np.ascontiguousarray(
        np.asarray(x, np.float32).reshape(B, C, N).astype(BF16NP))
    wqf = np.asarray(wq, np.float32)
    wkf = np.asarray(wk, np.float32)
    base = {
        "wqk8": np.ascontiguousarray((wqf.T @ wkf).astype(F8NP)),
        "wv8": np.ascontiguousarray(np.asarray(wv, np.float32).T.astype(F8NP)),
        "wo8": np.ascontiguousarray(np.asarray(wo, np.float32).T.astype(F8NP)),
        "bqk": wkf.T @ np.asarray(bq, np.float32),
        "beff": np.asarray(bo, np.float32) + np.asarray(wo, np.float32) @ np.asarray(bv, np.float32),
        "gamma": np.asarray(gn_gamma, np.float32),
        "beta": np.asarray(gn_beta, np.float32),
        "gmask": _gmask(),
    }
    in_maps = []
    for i in range(8):
        b, qc = i // 4, i % 4
        qoff = qc * NQ
        xrot = np.roll(xf[b], -qoff, axis=1)
        in_maps.append({**base, "xb": np.ascontiguousarray(xrot)})
    return in_maps


def _gmask():
    m = np.zeros((128, 128), np.float32)
    gs = 16
    for g in range(128 // gs):
        m[g * gs:(g + 1) * gs, g * gs:(g + 1) * gs] = 1.0 / gs
    return m


def assemble(results):
    full = np.zeros((2, C, N), np.float32)
    for i in range(8):
        b, qc = i // 4, i % 4
        full[b][:, qc * NQ:(qc + 1) * NQ] = results[i]["out"]
    return full.reshape(2, C, 64, 64)


_NC_CACHE = {}


def kernel(**inputs):
    import numpy as np
    x = np.asarray(inputs["x"], np.float32)
    if "build" not in _NC_CACHE:
        _NC_CACHE["build"] = build()
    nc = _NC_CACHE["build"]
    in_maps = make_in_maps(
        x, inputs["gn_gamma"], inputs["gn_beta"],
        inputs["wq"], inputs["bq"], inputs["wk"], inputs["bk"],
        inputs["wv"], inputs["bv"], inputs["wo"], inputs["bo"])
    res = run_bass_kernel_spmd(nc, in_maps, core_ids=list(range(8)))
    return assemble(res.results)
